# revision 11
# baseline (speedup 1.0000x reference)
"""AFTLocal kernel for 8 TRN2 NeuronCores.

Math: the reference's numerator/denominator = (dw*exp_k*v)/(dw*exp_k) = v
elementwise (all factors finite and > 0), so the module reduces exactly to

    out = (sigmoid(X @ Wq + bq) * (X @ Wv + bv)) @ Wo + bo

Sharding: data-parallel over batch. Each of the 8 cores processes 8 batches
(1024 tokens) with replicated weights; no collectives. The per-core
embedding shard is laid out [D_model, tokens] (transposed) when fed to the
NEFF so the contraction dim lands on SBUF partitions directly; all compute
(bf16 cast, matmuls, sigmoid, elementwise, f32 output) runs on-chip.

Per-core pipeline (bf16 matmuls, f32 PSUM accumulate):
  - cast-DMA (f32->bf16 during SWDGE) XT chunks + Wq/Wv/Wo chunks into SBUF
  - stage 1: QT/VT = Wq/Wv-chunk.T @ XT (accumulate over 8 k-chunks),
             fused epilogue HT = sigmoid(QT + bq) * (VT + bv) in bf16
  - stage 2: out tile = sum_k HT[k]-block.T @ Wo[k] + bo -> f32 -> DMA out
"""

import numpy as np

B, S, DM, DI = 64, 128, 1024, 1024
NCORES = 8
BL = B // NCORES          # batches per core
T = BL * S                # tokens per core = 1024
P = 128                   # partitions
KC = DM // P              # 8 contraction chunks
NT = T // P               # 8 token tiles of 128
NF = 512                  # matmul moving free dim (one PSUM bank of f32)
TN = T // NF              # 2 token blocks of 512
DN = DM // NF             # 2 output column blocks of 512

_CACHE = {}


# walrus in this container only supports 1 sync-wait per instruction for
# several ISA structs; Tile emits up to one wait per logical proc. Split
# excess waits into a chain of single-wait NoOps on the same engine
# (same-engine program order makes this equivalent).
def _split_waits(nc):
    from concourse import mybir

    for f in nc.m.functions:
        for b in f.blocks:
            new = []
            changed = False
            for inst in b.instructions:
                si = getattr(inst, "sync_info", None)
                limit = 1
                if si is not None and len(si.on_wait) > limit:
                    waits = list(si.on_wait)
                    extra, keep = waits[:-limit], waits[-limit:]
                    for i, w in enumerate(extra):
                        new.append(mybir.InstNoOp(
                            name=f"{inst.name}-wsplit{i}", ins=[], outs=[],
                            engine=inst.engine,
                            sync_info=mybir.SyncInfo(on_wait=[w], on_update=[]),
                        ))
                    inst.sync_info = mybir.SyncInfo(
                        on_wait=keep, on_update=list(si.on_update))
                    changed = True
                new.append(inst)
            if changed:
                b.instructions = new


def _build():
    import concourse.bass as bass
    import concourse.tile as tile
    from concourse import mybir
    from contextlib import ExitStack

    f32 = mybir.dt.float32
    bf16 = mybir.dt.bfloat16
    Act = mybir.ActivationFunctionType
    Alu = mybir.AluOpType

    nc = bass.Bass("TRN2")
    xt_d = nc.dram_tensor("xt", [DM, T], f32, kind="ExternalInput")
    wq_d = nc.dram_tensor("wq", [DM, DI], f32, kind="ExternalInput")
    wv_d = nc.dram_tensor("wv", [DM, DI], f32, kind="ExternalInput")
    wo_d = nc.dram_tensor("wo", [DI, DM], f32, kind="ExternalInput")
    bq_d = nc.dram_tensor("bq", [1, DI], f32, kind="ExternalInput")
    bv_d = nc.dram_tensor("bv", [1, DI], f32, kind="ExternalInput")
    bo_d = nc.dram_tensor("bo", [1, DM], f32, kind="ExternalInput")
    out_d = nc.dram_tensor("out", [T, DM], f32, kind="ExternalOutput")

    with ExitStack() as ctx:
        tc = ctx.enter_context(tile.TileContext(nc))
        consts = ctx.enter_context(tc.tile_pool(name="consts", bufs=1))
        wpool = ctx.enter_context(tc.tile_pool(name="weights", bufs=1))
        xtp = ctx.enter_context(tc.tile_pool(name="xt", bufs=1))
        htp = ctx.enter_context(tc.tile_pool(name="ht", bufs=1))
        stage = ctx.enter_context(tc.tile_pool(name="stage", bufs=4))
        sigp = ctx.enter_context(tc.tile_pool(name="sig", bufs=3))
        opool = ctx.enter_context(tc.tile_pool(name="opool", bufs=3))
        psum = ctx.enter_context(tc.tile_pool(name="psum", bufs=8, space="PSUM"))

        # ---- constants ----
        ones = consts.tile([1, NF], bf16)
        nc.vector.memset(ones, 1.0)

        # bias rows (bf16, cast during SWDGE DMA)
        bq_row = consts.tile([1, DI], bf16)
        bv_row = consts.tile([1, DI], bf16)
        nc.gpsimd.dma_start(out=bq_row, in_=bq_d[:, :])
        nc.gpsimd.dma_start(out=bv_row, in_=bv_d[:, :])

        # per-partition bias columns via K=1 matmuls (row -> column transpose)
        bq_pp = consts.tile([P, KC], f32)
        bv_pp = consts.tile([P, KC], f32)
        for di in range(KC):
            pc = psum.tile([P, 1], f32, tag="ps")
            nc.tensor.matmul(pc, bq_row[:, di * P:(di + 1) * P], ones[:, :1],
                             start=True, stop=True)
            nc.vector.tensor_copy(bq_pp[:, di:di + 1], pc)
            pc2 = psum.tile([P, 1], f32, tag="ps")
            nc.tensor.matmul(pc2, bv_row[:, di * P:(di + 1) * P], ones[:, :1],
                             start=True, stop=True)
            nc.vector.tensor_copy(bv_pp[:, di:di + 1], pc2)

        # bo broadcast to all partitions (added into the output epilogue):
        # DMA with a stride-0 partition AP replicates the DRAM row 128x.
        bo_bc = consts.tile([P, DM], f32)
        bo_ap = bo_d[:, :]
        bo_bcast_src = bass.AP(tensor=bo_ap.tensor, offset=bo_ap.offset,
                               ap=[[0, P]] + list(bo_ap.ap)[1:])
        nc.gpsimd.dma_start(out=bo_bc, in_=bo_bcast_src)

        # ---- XT + weights: f32->bf16 cast during DMA (SWDGE) ----
        xt = [xtp.tile([P, T], bf16, tag=f"xt{k}", name=f"xt{k}")
              for k in range(KC)]
        wq_bf = [wpool.tile([P, DI], bf16, tag=f"wq{k}", name=f"wq_bf{k}")
                 for k in range(KC)]
        wv_bf = [wpool.tile([P, DI], bf16, tag=f"wv{k}", name=f"wv_bf{k}")
                 for k in range(KC)]
        wo_bf = [wpool.tile([P, DM], bf16, tag=f"wo{k}", name=f"wo_bf{k}")
                 for k in range(KC)]
        for k in range(KC):
            nc.gpsimd.dma_start(out=xt[k], in_=xt_d[k * P:(k + 1) * P, :])
            for src_d, dst in ((wq_d, wq_bf), (wv_d, wv_bf)):
                stg = stage.tile([P, DI], f32, tag="wstg")
                nc.sync.dma_start(out=stg, in_=src_d[k * P:(k + 1) * P, :])
                nc.vector.tensor_copy(dst[k], stg)
        for k in range(KC):
            stg = stage.tile([P, DI], f32, tag="wstg")
            nc.sync.dma_start(out=stg, in_=wo_d[k * P:(k + 1) * P, :])
            nc.vector.tensor_copy(wo_bf[k], stg)

        # ---- stage 1: HT[di] = sigmoid(QT + bq) * (VT + bv) ----
        ht = [htp.tile([P, T], bf16, tag=f"ht{k}", name=f"ht{k}")
              for k in range(KC)]
        for tn in range(TN):
            ts = slice(tn * NF, (tn + 1) * NF)
            for di in range(KC):
                ps_q = psum.tile([P, NF], f32, tag="ps")
                ps_v = psum.tile([P, NF], f32, tag="ps")
                for k in range(KC):
                    nc.tensor.matmul(ps_q, wq_bf[k][:, di * P:(di + 1) * P],
                                     xt[k][:, ts], start=(k == 0),
                                     stop=(k == KC - 1))
                for k in range(KC):
                    nc.tensor.matmul(ps_v, wv_bf[k][:, di * P:(di + 1) * P],
                                     xt[k][:, ts], start=(k == 0),
                                     stop=(k == KC - 1))
                sig = sigp.tile([P, NF], bf16, tag="sig")
                nc.scalar.activation(sig, ps_q, Act.Sigmoid,
                                     bias=bq_pp[:, di:di + 1])
                nc.vector.scalar_tensor_tensor(
                    out=ht[di][:, ts], in0=ps_v, scalar=bv_pp[:, di:di + 1],
                    in1=sig, op0=Alu.add, op1=Alu.mult)

        # ---- stage 2: out = HT.T @ Wo + bo ----
        for t in range(NT):
            rs = slice(t * P, (t + 1) * P)
            for n in range(DN):
                cs = slice(n * NF, (n + 1) * NF)
                ps_o = psum.tile([P, NF], f32, tag="ps")
                for k in range(KC):
                    nc.tensor.matmul(ps_o, ht[k][:, rs], wo_bf[k][:, cs],
                                     start=(k == 0), stop=(k == KC - 1))
                ob = opool.tile([P, NF], f32, tag="ob")
                nc.vector.tensor_tensor(out=ob, in0=ps_o, in1=bo_bc[:, cs],
                                        op=Alu.add)
                nc.sync.dma_start(out=out_d[rs, cs], in_=ob)

    _split_waits(nc)
    return nc


def _get_nc():
    if "nc" not in _CACHE:
        _CACHE["nc"] = _build()
    return _CACHE["nc"]


def run(inputs, trace=False):
    """inputs: dict with setup_inputs() keys (numpy). Returns (out, exec_time_ns)."""
    from concourse import bass_utils

    nc = _get_nc()
    x = np.ascontiguousarray(np.asarray(inputs["embeddings"], dtype=np.float32)
                             ).reshape(B * S, DM)
    wq = np.ascontiguousarray(np.asarray(inputs["Wq"], dtype=np.float32))
    wv = np.ascontiguousarray(np.asarray(inputs["Wv"], dtype=np.float32))
    wo = np.ascontiguousarray(np.asarray(inputs["Wo"], dtype=np.float32))
    bq = np.asarray(inputs["bq"], dtype=np.float32).reshape(1, DI)
    bv = np.asarray(inputs["bv"], dtype=np.float32).reshape(1, DI)
    bo = np.asarray(inputs["bo"], dtype=np.float32).reshape(1, DM)

    in_maps = []
    for c in range(NCORES):
        shard_t = np.ascontiguousarray(x[c * T:(c + 1) * T].T)  # [DM, T]
        in_maps.append({
            "xt": shard_t,
            "wq": wq, "wv": wv, "wo": wo,
            "bq": bq, "bv": bv, "bo": bo,
        })
    res = bass_utils.run_bass_kernel_spmd(
        nc, in_maps, core_ids=list(range(NCORES)), trace=trace)
    out = np.concatenate([r["out"] for r in res.results], axis=0)
    return out.reshape(B, S, DM).astype(np.float32), res.exec_time_ns


def kernel(**inputs):
    out, _ = run(inputs, trace=False)
    return out


# revision 12
# speedup vs baseline: 1.1736x; 1.1736x over previous
"""AFTLocal kernel for 8 TRN2 NeuronCores.

Math: the reference's numerator/denominator = (dw*exp_k*v)/(dw*exp_k) = v
elementwise (all factors finite and > 0), so the module reduces exactly to

    out = (sigmoid(X @ Wq + bq) * (X @ Wv + bv)) @ Wo + bo

Sharding: data-parallel over batch. Each of the 8 cores processes 8 batches
(1024 tokens) with replicated weights; no collectives. The per-core
embedding shard is laid out [D_model, tokens] (transposed) when fed to the
NEFF so the contraction dim lands on SBUF partitions directly; all compute
(bf16 cast, matmuls, sigmoid, elementwise, f32 output) runs on-chip.

Per-core pipeline (bf16 matmuls, f32 PSUM accumulate):
  - cast-DMA (f32->bf16 during SWDGE) XT chunks + Wq/Wv/Wo chunks into SBUF
  - stage 1: QT/VT = Wq/Wv-chunk.T @ XT (accumulate over 8 k-chunks),
             fused epilogue HT = sigmoid(QT + bq) * (VT + bv) in bf16
  - stage 2: out tile = sum_k HT[k]-block.T @ Wo[k] + bo -> f32 -> DMA out
"""

import numpy as np

B, S, DM, DI = 64, 128, 1024, 1024
NCORES = 8
BL = B // NCORES          # batches per core
T = BL * S                # tokens per core = 1024
P = 128                   # partitions
KC = DM // P              # 8 contraction chunks
NT = T // P               # 8 token tiles of 128
NF = 512                  # matmul moving free dim (one PSUM bank of f32)
TN = T // NF              # 2 token blocks of 512
DN = DM // NF             # 2 output column blocks of 512

_CACHE = {}


# walrus in this container only supports 1 sync-wait per instruction for
# several ISA structs; Tile emits up to one wait per logical proc. Split
# excess waits into a chain of single-wait NoOps on the same engine
# (same-engine program order makes this equivalent).
def _split_waits(nc):
    from concourse import mybir

    for f in nc.m.functions:
        for b in f.blocks:
            new = []
            changed = False
            for inst in b.instructions:
                si = getattr(inst, "sync_info", None)
                limit = 1
                if si is not None and len(si.on_wait) > limit:
                    waits = list(si.on_wait)
                    extra, keep = waits[:-limit], waits[-limit:]
                    for i, w in enumerate(extra):
                        new.append(mybir.InstNoOp(
                            name=f"{inst.name}-wsplit{i}", ins=[], outs=[],
                            engine=inst.engine,
                            sync_info=mybir.SyncInfo(on_wait=[w], on_update=[]),
                        ))
                    inst.sync_info = mybir.SyncInfo(
                        on_wait=keep, on_update=list(si.on_update))
                    changed = True
                new.append(inst)
            if changed:
                b.instructions = new


def _build():
    import concourse.bass as bass
    import concourse.tile as tile
    from concourse import mybir
    from contextlib import ExitStack

    f32 = mybir.dt.float32
    bf16 = mybir.dt.bfloat16
    Act = mybir.ActivationFunctionType
    Alu = mybir.AluOpType

    nc = bass.Bass("TRN2")
    xt_d = nc.dram_tensor("xt", [DM, T], f32, kind="ExternalInput")
    wq_d = nc.dram_tensor("wq", [DM, DI], f32, kind="ExternalInput")
    wv_d = nc.dram_tensor("wv", [DM, DI], f32, kind="ExternalInput")
    wo_d = nc.dram_tensor("wo", [DI, DM], f32, kind="ExternalInput")
    bq_d = nc.dram_tensor("bq", [1, DI], f32, kind="ExternalInput")
    bv_d = nc.dram_tensor("bv", [1, DI], f32, kind="ExternalInput")
    bo_d = nc.dram_tensor("bo", [1, DM], f32, kind="ExternalInput")
    out_d = nc.dram_tensor("out", [T, DM], f32, kind="ExternalOutput")

    with ExitStack() as ctx:
        tc = ctx.enter_context(tile.TileContext(nc))
        consts = ctx.enter_context(tc.tile_pool(name="consts", bufs=1))
        wpool = ctx.enter_context(tc.tile_pool(name="weights", bufs=1))
        xtp = ctx.enter_context(tc.tile_pool(name="xt", bufs=1))
        htp = ctx.enter_context(tc.tile_pool(name="ht", bufs=1))
        stage = ctx.enter_context(tc.tile_pool(name="stage", bufs=4))
        sigp = ctx.enter_context(tc.tile_pool(name="sig", bufs=16))
        opool = ctx.enter_context(tc.tile_pool(name="opool", bufs=3))
        psum = ctx.enter_context(tc.tile_pool(name="psum", bufs=8, space="PSUM"))

        # ---- constants ----
        ones = consts.tile([1, NF], bf16)
        nc.vector.memset(ones, 1.0)

        # bias rows (bf16, cast during SWDGE DMA)
        bq_row = consts.tile([1, DI], bf16)
        bv_row = consts.tile([1, DI], bf16)
        nc.gpsimd.dma_start(out=bq_row, in_=bq_d[:, :])
        nc.gpsimd.dma_start(out=bv_row, in_=bv_d[:, :])

        # per-partition bias columns via K=1 matmuls (row -> column transpose)
        bq_pp = consts.tile([P, KC], f32)
        bv_pp = consts.tile([P, KC], f32)
        for di in range(KC):
            pc = psum.tile([P, 1], f32, tag="ps")
            nc.tensor.matmul(pc, bq_row[:, di * P:(di + 1) * P], ones[:, :1],
                             start=True, stop=True)
            nc.vector.tensor_copy(bq_pp[:, di:di + 1], pc)
            pc2 = psum.tile([P, 1], f32, tag="ps")
            nc.tensor.matmul(pc2, bv_row[:, di * P:(di + 1) * P], ones[:, :1],
                             start=True, stop=True)
            nc.vector.tensor_copy(bv_pp[:, di:di + 1], pc2)

        # bo broadcast to all partitions (added into the output epilogue):
        # DMA with a stride-0 partition AP replicates the DRAM row 128x.
        bo_bc = consts.tile([P, DM], f32)
        bo_ap = bo_d[:, :]
        bo_bcast_src = bass.AP(tensor=bo_ap.tensor, offset=bo_ap.offset,
                               ap=[[0, P]] + list(bo_ap.ap)[1:])
        nc.gpsimd.dma_start(out=bo_bc, in_=bo_bcast_src)

        # ---- XT + weights: f32->bf16 cast during DMA (SWDGE) ----
        xt = [xtp.tile([P, T], bf16, tag=f"xt{k}", name=f"xt{k}")
              for k in range(KC)]
        wq_bf = [wpool.tile([P, DI], bf16, tag=f"wq{k}", name=f"wq_bf{k}")
                 for k in range(KC)]
        wv_bf = [wpool.tile([P, DI], bf16, tag=f"wv{k}", name=f"wv_bf{k}")
                 for k in range(KC)]
        wo_bf = [wpool.tile([P, DM], bf16, tag=f"wo{k}", name=f"wo_bf{k}")
                 for k in range(KC)]
        # DMA order: xt first (needed by every group), then wq (q-pass can
        # go dense early), then wv, then wo. wq/wv cast on DVE in the same
        # order; wo cast on GpSimd (idle post-prologue) so the DVE FIFO never
        # blocks the v-pass epilogues behind wo.
        for k in range(KC):
            nc.gpsimd.dma_start(out=xt[k], in_=xt_d[k * P:(k + 1) * P, :])
        wq_stg = []
        for k in range(KC):
            stg = stage.tile([P, DI], f32, tag="wstgq", name=f"wqs{k}")
            nc.sync.dma_start(out=stg, in_=wq_d[k * P:(k + 1) * P, :])
            nc.vector.tensor_copy(wq_bf[k], stg)
        wv_stg = []
        for k in range(KC):
            stg = stage.tile([P, DI], f32, tag="wstgv", name=f"wvs{k}")
            nc.sync.dma_start(out=stg, in_=wv_d[k * P:(k + 1) * P, :])
            wv_stg.append(stg)
        wo_stg = []
        for k in range(KC):
            stg = stage.tile([P, DI], f32, tag="wstgo", name=f"wos{k}")
            nc.sync.dma_start(out=stg, in_=wo_d[k * P:(k + 1) * P, :])
            wo_stg.append(stg)
        for k in range(KC):
            nc.vector.tensor_copy(wv_bf[k], wv_stg[k])
        for k in range(KC):
            nc.gpsimd.tensor_copy(wo_bf[k], wo_stg[k])

        # ---- stage 1, q-pass: sig = sigmoid(QT + bq) for all tiles ----
        ht = [htp.tile([P, T], bf16, tag=f"ht{k}", name=f"ht{k}")
              for k in range(KC)]
        sigs = {}
        for tn in range(TN):
            ts = slice(tn * NF, (tn + 1) * NF)
            for di in range(KC):
                ps_q = psum.tile([P, NF], f32, tag="ps")
                for k in range(KC):
                    nc.tensor.matmul(ps_q, wq_bf[k][:, di * P:(di + 1) * P],
                                     xt[k][:, ts], start=(k == 0),
                                     stop=(k == KC - 1))
                sig = sigp.tile([P, NF], bf16, tag="sig",
                                name=f"sig{tn}_{di}")
                nc.scalar.activation(sig, ps_q, Act.Sigmoid,
                                     bias=bq_pp[:, di:di + 1])
                sigs[(tn, di)] = sig

        # ---- stage 1, v-pass: HT = sig * (VT + bv) ----
        for tn in range(TN):
            ts = slice(tn * NF, (tn + 1) * NF)
            for di in range(KC):
                ps_v = psum.tile([P, NF], f32, tag="ps")
                for k in range(KC):
                    nc.tensor.matmul(ps_v, wv_bf[k][:, di * P:(di + 1) * P],
                                     xt[k][:, ts], start=(k == 0),
                                     stop=(k == KC - 1))
                nc.vector.scalar_tensor_tensor(
                    out=ht[di][:, ts], in0=ps_v, scalar=bv_pp[:, di:di + 1],
                    in1=sigs[(tn, di)], op0=Alu.add, op1=Alu.mult)

        # ---- stage 2: out = HT.T @ Wo + bo ----
        for t in range(NT):
            rs = slice(t * P, (t + 1) * P)
            for n in range(DN):
                cs = slice(n * NF, (n + 1) * NF)
                ps_o = psum.tile([P, NF], f32, tag="ps")
                for k in range(KC):
                    nc.tensor.matmul(ps_o, ht[k][:, rs], wo_bf[k][:, cs],
                                     start=(k == 0), stop=(k == KC - 1))
                ob = opool.tile([P, NF], f32, tag="ob")
                nc.vector.tensor_tensor(out=ob, in0=ps_o, in1=bo_bc[:, cs],
                                        op=Alu.add)
                nc.sync.dma_start(out=out_d[rs, cs], in_=ob)

    _split_waits(nc)
    return nc


def _get_nc():
    if "nc" not in _CACHE:
        _CACHE["nc"] = _build()
    return _CACHE["nc"]


def run(inputs, trace=False):
    """inputs: dict with setup_inputs() keys (numpy). Returns (out, exec_time_ns)."""
    from concourse import bass_utils

    nc = _get_nc()
    x = np.ascontiguousarray(np.asarray(inputs["embeddings"], dtype=np.float32)
                             ).reshape(B * S, DM)
    wq = np.ascontiguousarray(np.asarray(inputs["Wq"], dtype=np.float32))
    wv = np.ascontiguousarray(np.asarray(inputs["Wv"], dtype=np.float32))
    wo = np.ascontiguousarray(np.asarray(inputs["Wo"], dtype=np.float32))
    bq = np.asarray(inputs["bq"], dtype=np.float32).reshape(1, DI)
    bv = np.asarray(inputs["bv"], dtype=np.float32).reshape(1, DI)
    bo = np.asarray(inputs["bo"], dtype=np.float32).reshape(1, DM)

    in_maps = []
    for c in range(NCORES):
        shard_t = np.ascontiguousarray(x[c * T:(c + 1) * T].T)  # [DM, T]
        in_maps.append({
            "xt": shard_t,
            "wq": wq, "wv": wv, "wo": wo,
            "bq": bq, "bv": bv, "bo": bo,
        })
    res = bass_utils.run_bass_kernel_spmd(
        nc, in_maps, core_ids=list(range(NCORES)), trace=trace)
    out = np.concatenate([r["out"] for r in res.results], axis=0)
    return out.reshape(B, S, DM).astype(np.float32), res.exec_time_ns


def kernel(**inputs):
    out, _ = run(inputs, trace=False)
    return out
